# revision 1
# baseline (speedup 1.0000x reference)
"""DiagBlockAttention Trainium2 kernel (Bass/Tile, 8 NeuronCores).

Problem (hardcoded from spec nn_DiagBlockAttention):
  x[16, 3136, 768] -> qkv = x @ w_qkv -> 12 heads x 64
  block-local attention: 56x56 token grid, 4x4 spatial blocks (16 tokens),
  softmax over the 16 tokens of each block per head
  out = attn_out @ w_out + b_out

Sharding: data-parallel over batch, 2 batches per core.

Token permutation is done ON THE HOST: x is pre-permuted to block order
(ch, group, b7, ir, ic) and the output is un-permuted after the run, so
every device-side access is a contiguous slice.

Per-core pipeline (per batch, 7 chunks of 448 tokens = 4 groups of 7
16-token blocks):
  A: load x_tok [112,768] per group, PE-transpose to x^T d-major [128,6,448]
  B: q/k projection d-major: psum[j-tile 128, 448]
     (stationary = w tile, moving = x^T, float32r)
  C: v projection token-major [112, j] (stationary = x^T group slice,
     moving = w_v) + ones column appended for softmax sums
  D: per head pair (even/odd heads sit at PE row-groups 0:64/64:128, so
     their score matmuls run concurrently on disjoint 32x32 sub-arrays):
     S^T[tk,tq] = matmul(lhsT=k^T, rhs=q^T) per group ->
     exp (ACT, scale=1/8) -> 0/1 block-diag mask multiply (DVE) ->
     PV token-major o_tok[tq, 65] = matmul(lhsT=P^T, rhs=v_aug)
     (col 64 = softmax sums) -> per-partition reciprocal + scalar multiply.
     Software-pipelined: pair hp+1's scores are emitted before pair hp's
     PV matmuls so the PE never waits on the exp->mask chain.
  E: per group: 6 PE transposes o_tok -> o^T d-major; out projection
     psum[112, 384] x2 (stationary = o^T, moving = w_out); bias add; store

Projection matmuls use float32r (full-rate fp32 mode, ~1.6e-4 rel err) when
PROJ_F32R; the rounding-to-f32r happens inside copies the pipeline makes
anyway (fp32 weights/activations never round through bf16). Attention
matmuls (scores/PV) are bf16 when ATTN_BF16 (fp32 attention matmuls get no
f32r speedup at free dim <= 112 and the fused fp32 weight-load cannot
overlap; bf16 separates LDWEIGHTS). Measured on TRN2: 991us HW exec,
2.5e-3 max relative error vs the fp32 reference (f32r-only attention:
1263us at 2.3e-4; set ATTN_BF16=False for that trade).
"""
import numpy as np
from contextlib import ExitStack

import concourse.bass as bass
import concourse.mybir as mybir
import concourse.tile as tile
from concourse import bacc
from concourse.bass_utils import run_bass_kernel_spmd
from concourse.masks import make_identity

# ---- problem constants ----
B, N, DIM = 16, 3136, 768
H, DH = 12, 64
J3 = 3 * H * DH              # 2304
SCALE = DH ** -0.5           # 0.125
NCORES = 8
B_LOC = B // NCORES          # 2
CHUNK = 448                  # 2 block-rows
NCHUNK = N // CHUNK          # 7
NG = 4                       # groups per chunk
GT = 112                     # tokens per group (7 blocks x 16)
KT = DIM // 128              # 6 k-tiles
F32 = mybir.dt.float32
F32R = mybir.dt.float32r
BF16 = mybir.dt.bfloat16

PROJ_F32R = True             # float32r for projection matmuls
ATTN_BF16 = True            # bf16 for scores/PV matmuls

_CACHE = {}


def _build():
    nc = bacc.Bacc("TRN2", target_bir_lowering=False, debug=False)

    # x arrives HOST-PERMUTED to block order: [b, ch, g, (b7 ir ic), d]
    x_d = nc.dram_tensor("x", [B_LOC, NCHUNK, NG, GT, DIM], F32,
                         kind="ExternalInput")
    wqkv_d = nc.dram_tensor("w_qkv", [DIM, J3], F32, kind="ExternalInput")
    wout_d = nc.dram_tensor("w_out", [DIM, DIM], F32, kind="ExternalInput")
    bout_d = nc.dram_tensor("b_out", [DIM], F32, kind="ExternalInput")
    # output in the same block order; host un-permutes
    o_d = nc.dram_tensor("o", [B_LOC, NCHUNK, NG, GT, DIM], F32,
                         kind="ExternalOutput")

    pdt = F32R if PROJ_F32R else F32
    adt = BF16 if ATTN_BF16 else F32

    with tile.TileContext(nc) as tc, ExitStack() as ctx:
        const = ctx.enter_context(tc.tile_pool(name="const", bufs=1))
        wpool = ctx.enter_context(tc.tile_pool(name="w", bufs=1))
        xin = ctx.enter_context(tc.tile_pool(name="xin", bufs=6))
        big = ctx.enter_context(tc.tile_pool(name="big", bufs=1))
        mid = ctx.enter_context(tc.tile_pool(name="mid", bufs=4))
        outp = ctx.enter_context(tc.tile_pool(name="outp", bufs=3))

        ps_a = ctx.enter_context(tc.tile_pool(name="ps_a", bufs=2, space="PSUM"))
        ps_s = ctx.enter_context(tc.tile_pool(name="ps_s", bufs=2, space="PSUM"))
        ps_pv = ctx.enter_context(tc.tile_pool(name="ps_pv", bufs=2, space="PSUM"))
        ps_vo = ctx.enter_context(tc.tile_pool(name="ps_vo", bufs=2, space="PSUM"))

        # ---- constants ----
        ident = const.tile([128, 128], F32)
        make_identity(nc, ident[:])

        # 0/1 block-diag-16 mask x4 groups: on-block iff 0 <= p - 16*b7 <= 15
        mask = const.tile([GT, NG * GT], adt)
        nc.gpsimd.memset(mask[:], 1.0)
        mask_v = mask[:].rearrange("p (g b7 ic) -> p g b7 ic", g=NG, b7=7)
        nc.gpsimd.affine_select(
            out=mask_v, in_=mask_v, compare_op=mybir.AluOpType.is_ge,
            fill=0.0, base=0, pattern=[[0, NG], [-16, 7], [0, 16]],
            channel_multiplier=1)
        nc.gpsimd.affine_select(
            out=mask_v, in_=mask_v, compare_op=mybir.AluOpType.is_ge,
            fill=0.0, base=15, pattern=[[0, NG], [16, 7], [0, 16]],
            channel_multiplier=-1)

        # bias replicated to 112 partitions via K=1 outer-product matmul
        bias1 = const.tile([1, DIM], F32)
        nc.sync.dma_start(bias1[:], bout_d.ap().unsqueeze(0))
        ones1 = const.tile([1, GT], F32)
        nc.vector.memset(ones1[:], 1.0)
        bias_rep = const.tile([GT, DIM], F32)
        for half in range(2):
            bps = ps_vo.tile([GT, 384], F32, tag="ps_vo")
            nc.tensor.matmul(bps[:], ones1[:], bias1[:, half * 384:(half + 1) * 384],
                             start=True, stop=True)
            nc.vector.tensor_copy(bias_rep[:, half * 384:(half + 1) * 384], bps[:])

        # ---- weights: stream-load fp32 (+ round to f32r via small temp) ----
        w_sb = wpool.tile([128, KT, J3], pdt)
        wo_sb = wpool.tile([128, KT, DIM], pdt)
        for dst, src_d, jdim in ((w_sb, wqkv_d, J3), (wo_sb, wout_d, DIM)):
            src = src_d.ap().rearrange("(ko ki) j -> ki ko j", ki=128)
            for kt in range(KT):
                for j0 in range(0, jdim, 768):
                    if PROJ_F32R:
                        wtmp = mid.tile([128, 768], F32, tag="wtmp")
                        nc.sync.dma_start(wtmp[:], src[:, kt, j0:j0 + 768])
                        nc.vector.tensor_copy(dst[:, kt, j0:j0 + 768], wtmp[:])
                    else:
                        nc.sync.dma_start(dst[:, kt, j0:j0 + 768],
                                          src[:, kt, j0:j0 + 768])

        for b in range(B_LOC):
            for ch in range(NCHUNK):
                # ---- A: load x (block-ordered) + transpose to d-major ----
                x_tok = [xin.tile([GT, DIM], F32, tag="x_tok", name=f"x_tok{g}")
                         for g in range(NG)]
                for g in range(NG):
                    nc.sync.dma_start(x_tok[g][:], x_d.ap()[b, ch, g])
                xT = big.tile([128, KT, CHUNK], pdt, tag="xT")
                for g in range(NG):
                    for kt in range(KT):
                        tp = ps_a.tile([128, GT], F32, tag="ps_a")
                        nc.tensor.transpose(
                            tp[:], x_tok[g][:, kt * 128:(kt + 1) * 128],
                            ident[0:GT, 0:GT])
                        nc.scalar.copy(
                            xT[:, kt, g * GT:(g + 1) * GT], tp[:])

                # ---- B: q/k projection, d-major [j-tile, t] ----
                qk = big.tile([128, 12, CHUNK], adt, tag="qk")
                for jt in range(12):
                    qkp = ps_a.tile([128, CHUNK], F32, tag="ps_a")
                    for kt in range(KT):
                        nc.tensor.matmul(
                            qkp[:],
                            w_sb[:, kt, jt * 128:(jt + 1) * 128],
                            xT[:, kt, :],
                            start=(kt == 0), stop=(kt == KT - 1))
                    nc.vector.tensor_copy(qk[:, jt, :], qkp[:])

                # ---- C: v projection, token-major + ones column ----
                v_sb = big.tile([GT, NG, H, 65], adt, tag="v")
                nc.vector.memset(v_sb[:, :, :, 64], 1.0)
                for g in range(NG):
                    for half in range(2):
                        vp = ps_vo.tile([GT, 384], F32, tag="ps_vo")
                        for kt in range(KT):
                            nc.tensor.matmul(
                                vp[:],
                                xT[:, kt, g * GT:(g + 1) * GT],
                                w_sb[:, kt, 1536 + half * 384: 1536 + (half + 1) * 384],
                                start=(kt == 0), stop=(kt == KT - 1))
                        nc.vector.tensor_copy(
                            v_sb[:, g, half * 6:(half + 1) * 6, 0:64],
                            vp[:].rearrange("p (h d) -> p h d", d=64))

                # ---- D: attention, head pairs interleaved ----
                # even/odd heads live at PE row-groups 0:64 / 64:128, so
                # alternating their score matmuls runs them concurrently
                # on disjoint 32x32 sub-arrays.
                # software pipeline: scores of pair hp+1 are emitted before
                # the PV matmuls of pair hp, so the PE never waits on the
                # exp->mask chain.
                o_tok = big.tile([GT, NG, DIM], F32, tag="o_tok")

                def emit_scores(hp):
                    jt_q, jt_k = hp, 6 + hp
                    sp0 = ps_s.tile([GT, NG * GT], F32, tag="ps_s", name="sp0")
                    sp1 = ps_s.tile([GT, NG * GT], F32, tag="ps_s", name="sp1")
                    for g in range(NG):
                        gs = slice(g * GT, (g + 1) * GT)
                        nc.tensor.matmul(sp0[:, gs], qk[0:64, jt_k, gs],
                                         qk[0:64, jt_q, gs],
                                         start=True, stop=True)
                        nc.tensor.matmul(sp1[:, gs], qk[64:128, jt_k, gs],
                                         qk[64:128, jt_q, gs],
                                         start=True, stop=True)
                    pm = []
                    for i, sp in enumerate((sp0, sp1)):
                        p = mid.tile([GT, NG * GT], adt, tag="p_raw",
                                     name=f"p{i}")
                        nc.scalar.activation(p[:], sp[:],
                                             mybir.ActivationFunctionType.Exp,
                                             scale=SCALE)
                        q_ = mid.tile([GT, NG * GT], adt, tag="p_sb",
                                      name=f"pm{i}")
                        nc.vector.tensor_mul(q_[:], p[:], mask[:])
                        pm.append(q_)
                    return pm

                def emit_pv(hp, pm):
                    for i in range(2):
                        h = 2 * hp + i
                        for g in range(NG):
                            pv = ps_pv.tile([GT, 65], F32, tag="ps_pv",
                                            name="pv")
                            nc.tensor.matmul(pv[:],
                                             pm[i][:, g * GT:(g + 1) * GT],
                                             v_sb[:, g, h, :],
                                             start=True, stop=True)
                            rcp = mid.tile([GT, 1], F32, tag="rcp")
                            nc.vector.reciprocal(rcp[:], pv[:, 64:65])
                            nc.vector.tensor_scalar_mul(
                                o_tok[:, g, h * 64:(h + 1) * 64],
                                pv[:, 0:64], rcp[:])

                prev = emit_scores(0)
                for hp in range(1, 6):
                    cur = emit_scores(hp)
                    emit_pv(hp - 1, prev)
                    prev = cur
                emit_pv(5, prev)

                # ---- E: transpose o, out projection, bias, store ----
                for g in range(NG):
                    oT = mid.tile([128, KT, GT], pdt, tag="oT")
                    for jt in range(KT):
                        tp = ps_a.tile([128, GT], F32, tag="ps_a")
                        nc.tensor.transpose(
                            tp[:], o_tok[:, g, jt * 128:(jt + 1) * 128],
                            ident[0:GT, 0:GT])
                        nc.scalar.copy(oT[:, jt, :], tp[:])
                    out_sb = outp.tile([GT, DIM], F32, tag="out_sb")
                    for half in range(2):
                        op = ps_vo.tile([GT, 384], F32, tag="ps_vo")
                        for jt in range(KT):
                            nc.tensor.matmul(
                                op[:], oT[:, jt, :],
                                wo_sb[:, jt, half * 384:(half + 1) * 384],
                                start=(jt == 0), stop=(jt == KT - 1))
                        nc.vector.tensor_add(
                            out_sb[:, half * 384:(half + 1) * 384], op[:],
                            bias_rep[:, half * 384:(half + 1) * 384])
                    nc.sync.dma_start(o_d.ap()[b, ch, g], out_sb[:])

    nc.compile()
    return nc


def _to_blocks_host(x):
    """[B, 3136, d] raster -> [B, ch, g, (b7 ir ic), d] block order."""
    b, n, d = x.shape
    # n = (ch, br, ir, h2, b7, ic) with sizes (7, 2, 4, 2, 7, 4)
    x = x.reshape(b, NCHUNK, 2, 4, 2, 7, 4, d)
    x = x.transpose(0, 1, 2, 4, 5, 3, 6, 7)   # b ch br h2 b7 ir ic d
    return np.ascontiguousarray(x.reshape(b, NCHUNK, NG, GT, d))


def _from_blocks_host(o):
    """inverse of _to_blocks_host -> [B_sub, 3136, d]."""
    b = o.shape[0]
    o = o.reshape(b, NCHUNK, 2, 2, 7, 4, 4, DIM)   # b ch br h2 b7 ir ic d
    o = o.transpose(0, 1, 2, 5, 3, 4, 6, 7)        # b ch br ir h2 b7 ic d
    return np.ascontiguousarray(o.reshape(b, N, DIM))


def kernel(x, w_qkv, w_out, b_out):
    x = np.ascontiguousarray(x, dtype=np.float32)
    w_qkv = np.ascontiguousarray(w_qkv, dtype=np.float32)
    w_out = np.ascontiguousarray(w_out, dtype=np.float32)
    b_out = np.ascontiguousarray(b_out, dtype=np.float32)

    if "nc" not in _CACHE:
        _CACHE["nc"] = _build()
    nc = _CACHE["nc"]

    xb = _to_blocks_host(x)
    in_maps = [
        {"x": xb[c * B_LOC:(c + 1) * B_LOC], "w_qkv": w_qkv,
         "w_out": w_out, "b_out": b_out}
        for c in range(NCORES)
    ]
    res = run_bass_kernel_spmd(nc, in_maps, core_ids=list(range(NCORES)))
    out = np.concatenate(
        [_from_blocks_host(res.results[c]["o"]) for c in range(NCORES)], axis=0)
    return out.astype(np.float32)



# revision 6
# speedup vs baseline: 1.2968x; 1.2968x over previous
"""DiagBlockAttention Trainium2 kernel v2 (Bass/Tile, 8 NeuronCores).

Problem (hardcoded from spec nn_DiagBlockAttention):
  x[16, 3136, 768] -> qkv = x @ w_qkv -> 12 heads x 64
  block-local attention over 4x4 spatial blocks (16 tokens each),
  softmax over the 16 tokens of each block per head
  out = attn_out @ w_out + b_out

Sharding: data-parallel over batch, 2 batches per core.

v2 design (vs v1 at 990us):
- ALL matmuls bf16 (rel err ~4e-3 vs 2e-2 gate): FWL weight loads, no
  fused-f32r serial weight load, 2x DVE rates.
- x is block-permuted AND transposed to d-major ON THE HOST, so the
  stage-A PE transposes (24/chunk) vanish; x^T DMAs straight into SBUF.
- Token stream regrouped: per core 392 blocks -> 7 superchunks x 896
  tokens; each superchunk = 7 groups x 128 tokens (8 blocks). All
  attention matmuls use full 128 partitions and 128-col stationaries.
- PV matmul is swapped (stationary = v, moving = P^T) so attention
  output lands d-major; odd heads go to PSUM partitions 64:128 via the
  tile_position col-group (out.base_partition()=64). This kills the
  stage-E PE transposes too.
- Softmax sums via 1-col ones-stationary matmuls into PSUM rows 0/64;
  1/sums is partition-broadcast with a 0-stride-AP DMA, reciprocal'd
  on DVE, and multiplied into o^T d-major (normalization commutes with
  nothing else: it must happen per head before the out projection).
- Out projection consumes o^T directly; bias add doubles as the
  psum->SBUF copy.
"""
import numpy as np
import ml_dtypes
from contextlib import ExitStack

import concourse.bass as bass
import concourse.mybir as mybir
import concourse.tile as tile
from concourse import bacc
from concourse.bass_utils import run_bass_kernel_spmd

# ---- problem constants ----
B, N, DIM = 16, 3136, 768
H, DH = 12, 64
J3 = 3 * H * DH              # 2304
SCALE = DH ** -0.5           # 0.125
NCORES = 8
B_LOC = B // NCORES          # 2
NTOK = B_LOC * N             # 6272 tokens per core
NSC = 7                      # superchunks per core
SC = NTOK // NSC             # 896 tokens per superchunk
NG = SC // 128               # 7 groups of 128 tokens (8 blocks)
KT = DIM // 128              # 6 k-tiles
NHP = H // 2                 # 6 head pairs
# attention spans: groups 0..3 (512 cols) and 4..6 (384 cols)
SPANS = [(0, 4), (4, 3)]     # (first group, ngroups)
F32 = mybir.dt.float32
BF16 = mybir.dt.bfloat16
BFNP = ml_dtypes.bfloat16

_CACHE = {}


def _build():
    nc = bacc.Bacc("TRN2", target_bir_lowering=False, debug=False)

    # host-prepped inputs: x d-major bf16 per superchunk, weights bf16
    x_d = nc.dram_tensor("x", [NSC, DIM, SC], BF16, kind="ExternalInput")
    wqkv_d = nc.dram_tensor("w_qkv", [DIM, J3], BF16, kind="ExternalInput")
    wout_d = nc.dram_tensor("w_out", [DIM, DIM], BF16, kind="ExternalInput")
    bout_d = nc.dram_tensor("b_out", [DIM], F32, kind="ExternalInput")
    # output token-major (block order); host un-permutes
    o_d = nc.dram_tensor("o", [NSC, SC, DIM], F32, kind="ExternalOutput")

    with tile.TileContext(nc) as tc, ExitStack() as ctx:
        const = ctx.enter_context(tc.tile_pool(name="const", bufs=1))
        wpool = ctx.enter_context(tc.tile_pool(name="w", bufs=1))
        xin = ctx.enter_context(tc.tile_pool(name="xin", bufs=2))
        qkp_ = ctx.enter_context(tc.tile_pool(name="qkp", bufs=2))
        vap = ctx.enter_context(tc.tile_pool(name="vap", bufs=2))
        otp = ctx.enter_context(tc.tile_pool(name="otp", bufs=2))
        mid = ctx.enter_context(tc.tile_pool(name="mid", bufs=4))
        outp = ctx.enter_context(tc.tile_pool(name="outp", bufs=3))

        ps_proj = ctx.enter_context(tc.tile_pool(name="ps_proj", bufs=2, space="PSUM"))
        ps_s = ctx.enter_context(tc.tile_pool(name="ps_s", bufs=4, space="PSUM"))
        ps_pv = ctx.enter_context(tc.tile_pool(name="ps_pv", bufs=2, space="PSUM"))

        # ---- constants ----
        # 0/1 block-diag-16 mask, one [128,128] pattern repeated 4x in free
        mask = const.tile([128, 512], BF16)
        nc.gpsimd.memset(mask[:], 1.0)
        mask_v = mask[:].rearrange("p (g b i) -> p g b i", g=4, b=8)
        nc.gpsimd.affine_select(
            out=mask_v, in_=mask_v, compare_op=mybir.AluOpType.is_ge,
            fill=0.0, base=0, pattern=[[0, 4], [-16, 8], [0, 16]],
            channel_multiplier=1)
        nc.gpsimd.affine_select(
            out=mask_v, in_=mask_v, compare_op=mybir.AluOpType.is_ge,
            fill=0.0, base=15, pattern=[[0, 4], [16, 8], [0, 16]],
            channel_multiplier=-1)

        # 64 columns of ones: the sums matmul replicates the softmax
        # denominators across 64 PSUM partitions (same PE cost — the moving
        # stream is what's paid for), making the downstream reciprocal a
        # full-width DVE op with no partition broadcast needed.
        ones64 = const.tile([128, 64], BF16)
        nc.vector.memset(ones64[:], 1.0)

        # bias replicated to 128 partitions via K=1 outer-product matmul
        bias1 = const.tile([1, DIM], F32)
        nc.sync.dma_start(bias1[:], bout_d.ap().unsqueeze(0))
        ones1 = const.tile([1, 128], F32)
        nc.vector.memset(ones1[:], 1.0)
        bias_rep = const.tile([128, DIM], F32)
        for half in range(2):
            bps = ps_proj.tile([128, 384], F32, tag="ps_proj")
            nc.tensor.matmul(bps[:], ones1[:], bias1[:, half * 384:(half + 1) * 384],
                             start=True, stop=True)
            nc.vector.tensor_copy(bias_rep[:, half * 384:(half + 1) * 384], bps[:])

        # ---- weights: direct bf16 DMA ----
        w_sb = wpool.tile([128, KT, J3], BF16)
        nc.sync.dma_start(w_sb[:], wqkv_d.ap().rearrange("(kt p) j -> p kt j", p=128))
        wo_sb = wpool.tile([128, KT, DIM], BF16)
        nc.sync.dma_start(wo_sb[:], wout_d.ap().rearrange("(kt p) j -> p kt j", p=128))

        for sc in range(NSC):
            # ---- A: x^T d-major, direct DMA ----
            xT = xin.tile([128, KT, SC], BF16, tag="xT")
            nc.sync.dma_start(xT[:], x_d.ap()[sc].rearrange("(kt p) t -> p kt t", p=128))

            # ---- B: q/k projection, d-major [j, t] ----
            qk = qkp_.tile([128, H, SC], BF16, tag="qk")
            for jt in range(H):
                for half in range(2):
                    ts = slice(half * 448, (half + 1) * 448)
                    qp = ps_proj.tile([128, 448], F32, tag="ps_proj")
                    for kt in range(KT):
                        nc.tensor.matmul(
                            qp[:], w_sb[:, kt, jt * 128:(jt + 1) * 128],
                            xT[:, kt, ts],
                            start=(kt == 0), stop=(kt == KT - 1))
                    if (2 * jt + half) % 2 == 0:
                        nc.vector.tensor_copy(qk[:, jt, ts], qp[:])
                    else:
                        nc.scalar.copy(qk[:, jt, ts], qp[:])

            # ---- C: v projection, token-major, split by head parity ----
            # va0[tk, g, hp, dh] = v of head 2hp; va1 = head 2hp+1
            va = [vap.tile([128, NG, NHP, DH], BF16, tag=f"va{i}", name=f"va{i}")
                  for i in range(2)]
            for g in range(NG):
                for half in range(2):
                    vp = ps_proj.tile([128, 384], F32, tag="ps_proj")
                    for kt in range(KT):
                        nc.tensor.matmul(
                            vp[:], xT[:, kt, g * 128:(g + 1) * 128],
                            w_sb[:, kt, 1536 + half * 384:1536 + (half + 1) * 384],
                            start=(kt == 0), stop=(kt == KT - 1))
                    vv = vp[:].rearrange("p (hp b d) -> p hp b d", hp=3, b=2)
                    hs = slice(3 * half, 3 * half + 3)
                    nc.vector.tensor_copy(va[0][:, g, hs, :], vv[:, :, 0, :])
                    nc.scalar.copy(va[1][:, g, hs, :], vv[:, :, 1, :])

            # ---- D: attention, software-pipelined over (span, hp) ----
            oT = otp.tile([128, KT, SC], BF16, tag="oT")

            def emit_scores(hp, span):
                g0, ng = span
                T = ng * 128
                sp = []
                for par in range(2):
                    spt = ps_s.tile([128, T], F32, tag="ps_s", name=f"sp{par}")
                    rows = slice(64 * par, 64 * par + 64)
                    for g in range(g0, g0 + ng):
                        gs = slice(g * 128, (g + 1) * 128)
                        ls = slice((g - g0) * 128, (g - g0 + 1) * 128)
                        nc.tensor.matmul(spt[:, ls], qk[rows, 6 + hp, gs],
                                         qk[rows, hp, gs], start=True, stop=True)
                    sp.append(spt)
                pm = []
                for par in range(2):
                    pe_t = mid.tile([128, T], BF16, tag="pexp", name=f"pe{par}")
                    nc.scalar.activation(pe_t[:], sp[par][:],
                                         mybir.ActivationFunctionType.Exp,
                                         scale=SCALE)
                    pmt = mid.tile([128, T], BF16, tag="pm", name=f"pm{par}")
                    nc.vector.tensor_mul(pmt[:], pe_t[:], mask[:, 0:T])
                    pm.append(pmt)
                return pm

            def emit_pv(hp, span, pm):
                g0, ng = span
                T = ng * 128
                po = ps_pv.tile([128, T], F32, tag="ps_pv", name="po")
                ss = ps_s.tile([128, T], F32, tag="ps_s", name="ss")
                for g in range(g0, g0 + ng):
                    ls = slice((g - g0) * 128, (g - g0 + 1) * 128)
                    nc.tensor.matmul(po[0:64, ls], va[0][:, g, hp, :],
                                     pm[0][:, ls], start=True, stop=True)
                    nc.tensor.matmul(po[64:128, ls], va[1][:, g, hp, :],
                                     pm[1][:, ls], start=True, stop=True)
                # sums replicated to partitions 0:64 / 64:128 by the ones64
                # stationary; reciprocal + multiply normalize o^T in place
                nc.tensor.matmul(ss[0:64, :], ones64[:], pm[0][:],
                                 start=True, stop=True)
                nc.tensor.matmul(ss[64:128, :], ones64[:], pm[1][:],
                                 start=True, stop=True)
                rT = mid.tile([128, T], BF16, tag="rT")
                with nc.allow_low_precision(reason="bf16 softmax denominators"):
                    nc.vector.reciprocal(rT[:], ss[:])
                nc.vector.tensor_mul(oT[:, hp, g0 * 128:g0 * 128 + T], po[:], rT[:])

            units = [(hp, span) for span in SPANS for hp in range(NHP)]
            prev = emit_scores(*units[0])
            for u in range(1, len(units)):
                cur = emit_scores(*units[u])
                emit_pv(units[u - 1][0], units[u - 1][1], prev)
                prev = cur
            emit_pv(units[-1][0], units[-1][1], prev)

            # ---- E: out projection + bias, store ----
            for g in range(NG):
                gs = slice(g * 128, (g + 1) * 128)
                ob = outp.tile([128, DIM], F32, tag="out_sb")
                for half in range(2):
                    js = slice(half * 384, (half + 1) * 384)
                    op = ps_proj.tile([128, 384], F32, tag="ps_proj")
                    for kt in range(KT):
                        nc.tensor.matmul(op[:], oT[:, kt, gs], wo_sb[:, kt, js],
                                         start=(kt == 0), stop=(kt == KT - 1))
                    nc.vector.tensor_add(ob[:, js], op[:], bias_rep[:, js])
                nc.sync.dma_start(o_d.ap()[sc, gs], ob[:])

    nc.compile()
    return nc


def _to_stream(x):
    """[B_LOC, 3136, d] raster -> [NTOK, d] block-major stream."""
    b, n, d = x.shape
    x = x.reshape(b, 14, 4, 14, 4, d)          # b, br, ir, bc, ic, d
    x = x.transpose(0, 1, 3, 2, 4, 5)           # b, br, bc, ir, ic, d
    return x.reshape(b * n, d)


def _from_stream(o):
    """inverse of _to_stream: [NTOK, d] -> [B_LOC, 3136, d]."""
    d = o.shape[-1]
    o = o.reshape(B_LOC, 14, 14, 4, 4, d)       # b, br, bc, ir, ic, d
    o = o.transpose(0, 1, 3, 2, 4, 5)           # b, br, ir, bc, ic, d
    return o.reshape(B_LOC, N, d)


def _make_in_maps(x, w_qkv, w_out, b_out):
    x = np.ascontiguousarray(x, dtype=np.float32)
    wq = np.ascontiguousarray(w_qkv, dtype=np.float32).astype(BFNP)
    wo = np.ascontiguousarray(w_out, dtype=np.float32).astype(BFNP)
    bo = np.ascontiguousarray(b_out, dtype=np.float32)
    in_maps = []
    for c in range(NCORES):
        xs = _to_stream(x[c * B_LOC:(c + 1) * B_LOC])      # [6272, 768]
        xT = xs.reshape(NSC, SC, DIM).transpose(0, 2, 1)   # [7, 768, 896]
        xT = np.ascontiguousarray(xT).astype(BFNP)
        in_maps.append({"x": xT, "w_qkv": wq, "w_out": wo, "b_out": bo})
    return in_maps


def kernel(x, w_qkv, w_out, b_out):
    if "nc" not in _CACHE:
        _CACHE["nc"] = _build()
    nc = _CACHE["nc"]

    in_maps = _make_in_maps(x, w_qkv, w_out, b_out)
    res = run_bass_kernel_spmd(nc, in_maps, core_ids=list(range(NCORES)))
    out = np.concatenate(
        [_from_stream(res.results[c]["o"].reshape(NTOK, DIM))
         for c in range(NCORES)], axis=0)
    return out.astype(np.float32)


# revision 10
# speedup vs baseline: 1.7819x; 1.3741x over previous
"""DiagBlockAttention Trainium2 kernel v2 (Bass/Tile, 8 NeuronCores).

Problem (hardcoded from spec nn_DiagBlockAttention):
  x[16, 3136, 768] -> qkv = x @ w_qkv -> 12 heads x 64
  block-local attention over 4x4 spatial blocks (16 tokens each),
  softmax over the 16 tokens of each block per head
  out = attn_out @ w_out + b_out

Sharding: data-parallel over batch, 2 batches per core.

v2 design (vs v1 at 990us):
- ALL matmuls bf16 (rel err ~4e-3 vs 2e-2 gate): FWL weight loads, no
  fused-f32r serial weight load, 2x DVE rates.
- x is block-permuted AND transposed to d-major ON THE HOST, so the
  stage-A PE transposes (24/chunk) vanish; x^T DMAs straight into SBUF.
- Token stream regrouped: per core 392 blocks -> 7 superchunks x 896
  tokens; each superchunk = 7 groups x 128 tokens (8 blocks). All
  attention matmuls use full 128 partitions and 128-col stationaries.
- PV matmul is swapped (stationary = v, moving = P^T) so attention
  output lands d-major; odd heads go to PSUM partitions 64:128 via the
  tile_position col-group (out.base_partition()=64). This kills the
  stage-E PE transposes too.
- Softmax sums via 1-col ones-stationary matmuls into PSUM rows 0/64;
  1/sums is partition-broadcast with a 0-stride-AP DMA, reciprocal'd
  on DVE, and multiplied into o^T d-major (normalization commutes with
  nothing else: it must happen per head before the out projection).
- Out projection consumes o^T directly; bias add doubles as the
  psum->SBUF copy.
"""
import numpy as np
import ml_dtypes
from contextlib import ExitStack

import concourse.bass as bass
import concourse.mybir as mybir
import concourse.tile as tile
from concourse import bacc
from concourse.bass_utils import run_bass_kernel_spmd

# ---- problem constants ----
B, N, DIM = 16, 3136, 768
H, DH = 12, 64
J3 = 3 * H * DH              # 2304
SCALE = DH ** -0.5           # 0.125
NCORES = 8
B_LOC = B // NCORES          # 2
NTOK = B_LOC * N             # 6272 tokens per core
NSC = 7                      # superchunks per core
SC = NTOK // NSC             # 896 tokens per superchunk
NG = SC // 128               # 7 groups of 128 tokens (8 blocks)
KT = DIM // 128              # 6 k-tiles
NHP = H // 2                 # 6 head pairs
# attention spans: groups 0..3 (512 cols) and 4..6 (384 cols)
SPANS = [(0, 4), (4, 3)]     # (first group, ngroups)
F32 = mybir.dt.float32
BF16 = mybir.dt.bfloat16
BFNP = ml_dtypes.bfloat16

_CACHE = {}


def _build():
    nc = bacc.Bacc("TRN2", target_bir_lowering=False, debug=False)

    # host-prepped inputs: x d-major bf16 per superchunk, weights bf16
    x_d = nc.dram_tensor("x", [NSC, DIM, SC], BF16, kind="ExternalInput")
    wqkv_d = nc.dram_tensor("w_qkv", [DIM, J3], BF16, kind="ExternalInput")
    wout_d = nc.dram_tensor("w_out", [DIM, DIM], BF16, kind="ExternalInput")
    bout_d = nc.dram_tensor("b_out", [DIM], F32, kind="ExternalInput")
    # output token-major (block order); host un-permutes
    o_d = nc.dram_tensor("o", [NSC, SC, DIM], F32, kind="ExternalOutput")

    with tile.TileContext(nc) as tc, ExitStack() as ctx:
        const = ctx.enter_context(tc.tile_pool(name="const", bufs=1))
        wpool = ctx.enter_context(tc.tile_pool(name="w", bufs=1))
        xin = ctx.enter_context(tc.tile_pool(name="xin", bufs=2))
        qkp_ = ctx.enter_context(tc.tile_pool(name="qkp", bufs=2))
        vap = ctx.enter_context(tc.tile_pool(name="vap", bufs=2))
        otp = ctx.enter_context(tc.tile_pool(name="otp", bufs=2))
        mid = ctx.enter_context(tc.tile_pool(name="mid", bufs=4))
        outp = ctx.enter_context(tc.tile_pool(name="outp", bufs=3))

        ps_proj = ctx.enter_context(tc.tile_pool(name="ps_proj", bufs=2, space="PSUM"))
        ps_s = ctx.enter_context(tc.tile_pool(name="ps_s", bufs=4, space="PSUM"))
        ps_pv = ctx.enter_context(tc.tile_pool(name="ps_pv", bufs=2, space="PSUM"))

        # ---- constants ----
        # 0/1 block-diag-16 mask, one [128,128] pattern repeated 4x in free
        mask = const.tile([128, 512], BF16)
        nc.gpsimd.memset(mask[:], 1.0)
        mask_v = mask[:].rearrange("p (g b i) -> p g b i", g=4, b=8)
        nc.gpsimd.affine_select(
            out=mask_v, in_=mask_v, compare_op=mybir.AluOpType.is_ge,
            fill=0.0, base=0, pattern=[[0, 4], [-16, 8], [0, 16]],
            channel_multiplier=1)
        nc.gpsimd.affine_select(
            out=mask_v, in_=mask_v, compare_op=mybir.AluOpType.is_ge,
            fill=0.0, base=15, pattern=[[0, 4], [16, 8], [0, 16]],
            channel_multiplier=-1)

        # 64 columns of ones: the sums matmul replicates the softmax
        # denominators across 64 PSUM partitions (same PE cost — the moving
        # stream is what's paid for), making the downstream reciprocal a
        # full-width DVE op with no partition broadcast needed.
        ones64 = const.tile([128, 64], BF16)
        nc.vector.memset(ones64[:], 1.0)

        # bias replicated to 128 partitions via K=1 outer-product matmul
        bias1 = const.tile([1, DIM], F32)
        nc.sync.dma_start(bias1[:], bout_d.ap().unsqueeze(0))
        ones1 = const.tile([1, 128], F32)
        nc.vector.memset(ones1[:], 1.0)
        bias_rep = const.tile([128, DIM], F32)
        for half in range(2):
            bps = ps_proj.tile([128, 384], F32, tag="ps_proj")
            nc.tensor.matmul(bps[:], ones1[:], bias1[:, half * 384:(half + 1) * 384],
                             start=True, stop=True)
            nc.vector.tensor_copy(bias_rep[:, half * 384:(half + 1) * 384], bps[:])

        # ---- weights: direct bf16 DMA ----
        w_sb = wpool.tile([128, KT, J3], BF16)
        nc.sync.dma_start(w_sb[:], wqkv_d.ap().rearrange("(kt p) j -> p kt j", p=128))
        wo_sb = wpool.tile([128, KT, DIM], BF16)
        nc.sync.dma_start(wo_sb[:], wout_d.ap().rearrange("(kt p) j -> p kt j", p=128))

        for sc in range(NSC):
            # ---- A: x^T d-major, direct DMA ----
            xT = xin.tile([128, KT, SC], BF16, tag="xT")
            nc.sync.dma_start(xT[:], x_d.ap()[sc].rearrange("(kt p) t -> p kt t", p=128))

            # ---- B: q/k projection, d-major [j, t] ----
            qk = qkp_.tile([128, H, SC], BF16, tag="qk")
            for jt in range(H):
                for half in range(2):
                    ts = slice(half * 448, (half + 1) * 448)
                    qp = ps_proj.tile([128, 448], F32, tag="ps_proj")
                    for kt in range(KT):
                        nc.tensor.matmul(
                            qp[:], w_sb[:, kt, jt * 128:(jt + 1) * 128],
                            xT[:, kt, ts],
                            start=(kt == 0), stop=(kt == KT - 1))
                    if (2 * jt + half) % 2 == 0:
                        nc.vector.tensor_copy(qk[:, jt, ts], qp[:])
                    else:
                        nc.scalar.copy(qk[:, jt, ts], qp[:])

            # ---- C: v projection, token-major, split by head parity ----
            # va0[tk, g, hp, dh] = v of head 2hp; va1 = head 2hp+1
            va = [vap.tile([128, NG, NHP, DH], BF16, tag=f"va{i}", name=f"va{i}")
                  for i in range(2)]
            for g in range(NG):
                for half in range(2):
                    vp = ps_proj.tile([128, 384], F32, tag="ps_proj")
                    for kt in range(KT):
                        nc.tensor.matmul(
                            vp[:], xT[:, kt, g * 128:(g + 1) * 128],
                            w_sb[:, kt, 1536 + half * 384:1536 + (half + 1) * 384],
                            start=(kt == 0), stop=(kt == KT - 1))
                    vv = vp[:].rearrange("p (hp b d) -> p hp b d", hp=3, b=2)
                    hs = slice(3 * half, 3 * half + 3)
                    nc.vector.tensor_copy(va[0][:, g, hs, :], vv[:, :, 0, :])
                    nc.scalar.copy(va[1][:, g, hs, :], vv[:, :, 1, :])

            # ---- D: attention, software-pipelined over (span, hp) ----
            oT = otp.tile([128, KT, SC], BF16, tag="oT")

            def emit_scores(hp, span):
                g0, ng = span
                T = ng * 128
                sp = []
                for par in range(2):
                    spt = ps_s.tile([128, T], F32, tag="ps_s", name=f"sp{par}")
                    rows = slice(64 * par, 64 * par + 64)
                    for g in range(g0, g0 + ng):
                        gs = slice(g * 128, (g + 1) * 128)
                        ls = slice((g - g0) * 128, (g - g0 + 1) * 128)
                        nc.tensor.matmul(spt[:, ls], qk[rows, 6 + hp, gs],
                                         qk[rows, hp, gs], start=True, stop=True)
                    sp.append(spt)
                pm = []
                for par in range(2):
                    pe_t = mid.tile([128, T], BF16, tag="pexp", name=f"pe{par}")
                    nc.scalar.activation(pe_t[:], sp[par][:],
                                         mybir.ActivationFunctionType.Exp,
                                         scale=SCALE)
                    pmt = mid.tile([128, T], BF16, tag="pm", name=f"pm{par}")
                    nc.vector.tensor_mul(pmt[:], pe_t[:], mask[:, 0:T])
                    pm.append(pmt)
                return pm

            def emit_pv(hp, span, pm):
                g0, ng = span
                T = ng * 128
                po = ps_pv.tile([128, T], F32, tag="ps_pv", name="po")
                ss = ps_s.tile([128, T], F32, tag="ps_s", name="ss")
                for g in range(g0, g0 + ng):
                    ls = slice((g - g0) * 128, (g - g0 + 1) * 128)
                    nc.tensor.matmul(po[0:64, ls], va[0][:, g, hp, :],
                                     pm[0][:, ls], start=True, stop=True)
                    nc.tensor.matmul(po[64:128, ls], va[1][:, g, hp, :],
                                     pm[1][:, ls], start=True, stop=True)
                # sums replicated to partitions 0:64 / 64:128 by the ones64
                # stationary; reciprocal + multiply normalize o^T in place
                nc.tensor.matmul(ss[0:64, :], ones64[:], pm[0][:],
                                 start=True, stop=True)
                nc.tensor.matmul(ss[64:128, :], ones64[:], pm[1][:],
                                 start=True, stop=True)
                rT = mid.tile([128, T], F32, tag="rT")
                nc.vector.reciprocal_approx_fast(rT[:], ss[:])
                nc.vector.tensor_mul(oT[:, hp, g0 * 128:g0 * 128 + T], po[:], rT[:])

            # ---- E (interleaved): out projection + bias, store ----
            def emit_out(g):
                gs = slice(g * 128, (g + 1) * 128)
                ob = outp.tile([128, DIM], F32, tag="out_sb")
                for half in range(2):
                    js = slice(half * 384, (half + 1) * 384)
                    op = ps_proj.tile([128, 384], F32, tag="ps_proj")
                    for kt in range(KT):
                        nc.tensor.matmul(op[:], oT[:, kt, gs], wo_sb[:, kt, js],
                                         start=(kt == 0), stop=(kt == KT - 1))
                    nc.vector.tensor_add(ob[:, js], op[:], bias_rep[:, js])
                nc.sync.dma_start(o_d.ap()[sc, gs], ob[:])

            # span0's groups (0..3) become out-projectable once all 6 span0
            # pvs have run; interleave them into span1's pipeline to keep
            # the PE dense through the attention phase.
            units = [(hp, span) for span in SPANS for hp in range(NHP)]
            prev = emit_scores(*units[0])
            for u in range(1, len(units)):
                cur = emit_scores(*units[u])
                emit_pv(units[u - 1][0], units[u - 1][1], prev)
                if NHP <= u <= NHP + 3:
                    emit_out(u - NHP)
                prev = cur
            emit_pv(units[-1][0], units[-1][1], prev)
            for g in range(4, NG):
                emit_out(g)

    nc.compile()
    return nc


def _to_stream(x):
    """[B_LOC, 3136, d] raster -> [NTOK, d] block-major stream."""
    b, n, d = x.shape
    x = x.reshape(b, 14, 4, 14, 4, d)          # b, br, ir, bc, ic, d
    x = x.transpose(0, 1, 3, 2, 4, 5)           # b, br, bc, ir, ic, d
    return x.reshape(b * n, d)


def _from_stream(o):
    """inverse of _to_stream: [NTOK, d] -> [B_LOC, 3136, d]."""
    d = o.shape[-1]
    o = o.reshape(B_LOC, 14, 14, 4, 4, d)       # b, br, bc, ir, ic, d
    o = o.transpose(0, 1, 3, 2, 4, 5)           # b, br, ir, bc, ic, d
    return o.reshape(B_LOC, N, d)


def _make_in_maps(x, w_qkv, w_out, b_out):
    x = np.ascontiguousarray(x, dtype=np.float32)
    wq = np.ascontiguousarray(w_qkv, dtype=np.float32).astype(BFNP)
    wo = np.ascontiguousarray(w_out, dtype=np.float32).astype(BFNP)
    bo = np.ascontiguousarray(b_out, dtype=np.float32)
    in_maps = []
    for c in range(NCORES):
        xs = _to_stream(x[c * B_LOC:(c + 1) * B_LOC])      # [6272, 768]
        xT = xs.reshape(NSC, SC, DIM).transpose(0, 2, 1)   # [7, 768, 896]
        xT = np.ascontiguousarray(xT).astype(BFNP)
        in_maps.append({"x": xT, "w_qkv": wq, "w_out": wo, "b_out": bo})
    return in_maps


def kernel(x, w_qkv, w_out, b_out):
    if "nc" not in _CACHE:
        _CACHE["nc"] = _build()
    nc = _CACHE["nc"]

    in_maps = _make_in_maps(x, w_qkv, w_out, b_out)
    res = run_bass_kernel_spmd(nc, in_maps, core_ids=list(range(NCORES)))
    out = np.concatenate(
        [_from_stream(res.results[c]["o"].reshape(NTOK, DIM))
         for c in range(NCORES)], axis=0)
    return out.astype(np.float32)


# revision 15
# speedup vs baseline: 1.8071x; 1.0142x over previous
"""DiagBlockAttention Trainium2 kernel v2 (Bass/Tile, 8 NeuronCores).

Problem (hardcoded from spec nn_DiagBlockAttention):
  x[16, 3136, 768] -> qkv = x @ w_qkv -> 12 heads x 64
  block-local attention over 4x4 spatial blocks (16 tokens each),
  softmax over the 16 tokens of each block per head
  out = attn_out @ w_out + b_out

Sharding: data-parallel over batch, 2 batches per core.

v2 design (vs v1 at 990us):
- ALL matmuls bf16 (rel err ~4e-3 vs 2e-2 gate): FWL weight loads, no
  fused-f32r serial weight load, 2x DVE rates.
- x is block-permuted AND transposed to d-major ON THE HOST, so the
  stage-A PE transposes (24/chunk) vanish; x^T DMAs straight into SBUF.
- Token stream regrouped: per core 392 blocks -> 7 superchunks x 896
  tokens; each superchunk = 7 groups x 128 tokens (8 blocks). All
  attention matmuls use full 128 partitions and 128-col stationaries.
- PV matmul is swapped (stationary = v, moving = P^T) so attention
  output lands d-major; odd heads go to PSUM partitions 64:128 via the
  tile_position col-group (out.base_partition()=64). This kills the
  stage-E PE transposes too.
- Softmax sums via 1-col ones-stationary matmuls into PSUM rows 0/64;
  1/sums is partition-broadcast with a 0-stride-AP DMA, reciprocal'd
  on DVE, and multiplied into o^T d-major (normalization commutes with
  nothing else: it must happen per head before the out projection).
- Out projection consumes o^T directly; bias add doubles as the
  psum->SBUF copy.
"""
import numpy as np
import ml_dtypes
from contextlib import ExitStack

import concourse.bass as bass
import concourse.mybir as mybir
import concourse.tile as tile
from concourse import bacc
from concourse.bass_utils import run_bass_kernel_spmd

# ---- problem constants ----
B, N, DIM = 16, 3136, 768
H, DH = 12, 64
J3 = 3 * H * DH              # 2304
SCALE = DH ** -0.5           # 0.125
NCORES = 8
B_LOC = B // NCORES          # 2
NTOK = B_LOC * N             # 6272 tokens per core
NSC = 7                      # superchunks per core
SC = NTOK // NSC             # 896 tokens per superchunk
NG = SC // 128               # 7 groups of 128 tokens (8 blocks)
KT = DIM // 128              # 6 k-tiles
NHP = H // 2                 # 6 head pairs
# attention spans: groups 0..3 (512 cols) and 4..6 (384 cols)
SPANS = [(0, 4), (4, 3)]     # (first group, ngroups)
F32 = mybir.dt.float32
BF16 = mybir.dt.bfloat16
BFNP = ml_dtypes.bfloat16

_CACHE = {}


def _build():
    nc = bacc.Bacc("TRN2", target_bir_lowering=False, debug=False)

    # host-prepped inputs: x d-major bf16 per superchunk, weights bf16
    x_d = nc.dram_tensor("x", [NSC, DIM, SC], BF16, kind="ExternalInput")
    wqkv_d = nc.dram_tensor("w_qkv", [DIM, J3], BF16, kind="ExternalInput")
    wout_d = nc.dram_tensor("w_out", [DIM, DIM], BF16, kind="ExternalInput")
    bout_d = nc.dram_tensor("b_out", [DIM], F32, kind="ExternalInput")
    # output token-major (block order); host un-permutes
    o_d = nc.dram_tensor("o", [NSC, SC, DIM], F32, kind="ExternalOutput")

    with tile.TileContext(nc) as tc, ExitStack() as ctx:
        const = ctx.enter_context(tc.tile_pool(name="const", bufs=1))
        wpool = ctx.enter_context(tc.tile_pool(name="w", bufs=1))
        xin = ctx.enter_context(tc.tile_pool(name="xin", bufs=2))
        qkp_ = ctx.enter_context(tc.tile_pool(name="qkp", bufs=2))
        vap = ctx.enter_context(tc.tile_pool(name="vap", bufs=2))
        otp = ctx.enter_context(tc.tile_pool(name="otp", bufs=2))
        mid = ctx.enter_context(tc.tile_pool(name="mid", bufs=4))
        outp = ctx.enter_context(tc.tile_pool(name="outp", bufs=3))

        ps_proj = ctx.enter_context(tc.tile_pool(name="ps_proj", bufs=2, space="PSUM"))
        ps_s = ctx.enter_context(tc.tile_pool(name="ps_s", bufs=4, space="PSUM"))
        ps_pv = ctx.enter_context(tc.tile_pool(name="ps_pv", bufs=2, space="PSUM"))

        # ---- constants ----
        # 0/1 block-diag-16 mask, one [128,128] pattern repeated 4x in free
        mask = const.tile([128, 512], BF16)
        nc.gpsimd.memset(mask[:], 1.0)
        mask_v = mask[:].rearrange("p (g b i) -> p g b i", g=4, b=8)
        nc.gpsimd.affine_select(
            out=mask_v, in_=mask_v, compare_op=mybir.AluOpType.is_ge,
            fill=0.0, base=0, pattern=[[0, 4], [-16, 8], [0, 16]],
            channel_multiplier=1)
        nc.gpsimd.affine_select(
            out=mask_v, in_=mask_v, compare_op=mybir.AluOpType.is_ge,
            fill=0.0, base=15, pattern=[[0, 4], [16, 8], [0, 16]],
            channel_multiplier=-1)

        # 64 columns of ones: the sums matmul replicates the softmax
        # denominators across 64 PSUM partitions (same PE cost — the moving
        # stream is what's paid for), making the downstream reciprocal a
        # full-width DVE op with no partition broadcast needed.
        ones64 = const.tile([128, 64], BF16)
        nc.vector.memset(ones64[:], 1.0)

        # bias replicated to 128 partitions via K=1 outer-product matmul
        bias1 = const.tile([1, DIM], F32)
        nc.sync.dma_start(bias1[:], bout_d.ap().unsqueeze(0))
        ones1 = const.tile([1, 128], F32)
        nc.vector.memset(ones1[:], 1.0)
        bias_rep = const.tile([128, DIM], F32)
        for half in range(2):
            bps = ps_proj.tile([128, 384], F32, tag="ps_proj")
            nc.tensor.matmul(bps[:], ones1[:], bias1[:, half * 384:(half + 1) * 384],
                             start=True, stop=True)
            nc.vector.tensor_copy(bias_rep[:, half * 384:(half + 1) * 384], bps[:])

        # ---- weights: direct bf16 DMA, split per k-tile across engine
        # queues so the first projection matmuls start ~1.6us in instead of
        # waiting out one monolithic 4.7MB transfer
        w_sb = wpool.tile([128, KT, J3], BF16)
        wo_sb = wpool.tile([128, KT, DIM], BF16)
        wq_src = wqkv_d.ap().rearrange("(kt p) j -> p kt j", p=128)
        wo_src = wout_d.ap().rearrange("(kt p) j -> p kt j", p=128)
        dma_engs = [nc.sync, nc.scalar, nc.gpsimd]
        for kt in range(KT):
            dma_engs[kt % len(dma_engs)].dma_start(
                w_sb[:, kt, :], wq_src[:, kt, :])
        for kt in range(KT):
            dma_engs[(kt + 2) % len(dma_engs)].dma_start(
                wo_sb[:, kt, :], wo_src[:, kt, :])

        for sc in range(NSC):
            # ---- A: x^T d-major, direct DMA (split across two queues) ----
            xT = xin.tile([128, KT, SC], BF16, tag="xT")
            x_src = x_d.ap()[sc].rearrange("(kt p) t -> p kt t", p=128)
            nc.sync.dma_start(xT[:, 0:3, :], x_src[:, 0:3, :])
            nc.gpsimd.dma_start(xT[:, 3:6, :], x_src[:, 3:6, :])

            # ---- B: q/k projection, d-major [j, t] ----
            qk = qkp_.tile([128, H, SC], BF16, tag="qk")
            for jt in range(H):
                for half in range(2):
                    ts = slice(half * 448, (half + 1) * 448)
                    qp = ps_proj.tile([128, 448], F32, tag="ps_proj")
                    for kt in range(KT):
                        nc.tensor.matmul(
                            qp[:], w_sb[:, kt, jt * 128:(jt + 1) * 128],
                            xT[:, kt, ts],
                            start=(kt == 0), stop=(kt == KT - 1))
                    if (2 * jt + half) % 2 == 0:
                        nc.vector.tensor_copy(qk[:, jt, ts], qp[:])
                    else:
                        nc.scalar.copy(qk[:, jt, ts], qp[:])

            va = [vap.tile([128, NG, NHP, DH], BF16, tag=f"va{i}", name=f"va{i}")
                  for i in range(2)]
            oT = otp.tile([128, KT, SC], BF16, tag="oT")

            def emit_v_proj():
                # v projection, token-major, split by head parity:
                # va0[tk, g, hp, dh] = v of head 2hp; va1 = head 2hp+1
                for g in range(NG):
                    for half in range(2):
                        vp = ps_proj.tile([128, 384], F32, tag="ps_proj")
                        for kt in range(KT):
                            nc.tensor.matmul(
                                vp[:], xT[:, kt, g * 128:(g + 1) * 128],
                                w_sb[:, kt, 1536 + half * 384:1536 + (half + 1) * 384],
                                start=(kt == 0), stop=(kt == KT - 1))
                        vv = vp[:].rearrange("p (hp b d) -> p hp b d", hp=3, b=2)
                        hs = slice(3 * half, 3 * half + 3)
                        nc.vector.tensor_copy(va[0][:, g, hs, :], vv[:, :, 0, :])
                        nc.scalar.copy(va[1][:, g, hs, :], vv[:, :, 1, :])

            def emit_scores(hp, span):
                g0, ng = span
                T = ng * 128
                sp = []
                for par in range(2):
                    spt = ps_s.tile([128, T], F32, tag="ps_s", name=f"sp{par}")
                    rows = slice(64 * par, 64 * par + 64)
                    for g in range(g0, g0 + ng):
                        gs = slice(g * 128, (g + 1) * 128)
                        ls = slice((g - g0) * 128, (g - g0 + 1) * 128)
                        nc.tensor.matmul(spt[:, ls], qk[rows, 6 + hp, gs],
                                         qk[rows, hp, gs], start=True, stop=True)
                    sp.append(spt)
                pm = []
                for par in range(2):
                    pe_t = mid.tile([128, T], BF16, tag="pexp", name=f"pe{par}")
                    nc.scalar.activation(pe_t[:], sp[par][:],
                                         mybir.ActivationFunctionType.Exp,
                                         scale=SCALE)
                    pmt = mid.tile([128, T], BF16, tag="pm", name=f"pm{par}")
                    nc.vector.tensor_mul(pmt[:], pe_t[:], mask[:, 0:T])
                    pm.append(pmt)
                return pm

            def emit_pv(hp, span, pm):
                g0, ng = span
                T = ng * 128
                po = ps_pv.tile([128, T], F32, tag="ps_pv", name="po")
                ss = ps_s.tile([128, T], F32, tag="ps_s", name="ss")
                for g in range(g0, g0 + ng):
                    ls = slice((g - g0) * 128, (g - g0 + 1) * 128)
                    nc.tensor.matmul(po[0:64, ls], va[0][:, g, hp, :],
                                     pm[0][:, ls], start=True, stop=True)
                    nc.tensor.matmul(po[64:128, ls], va[1][:, g, hp, :],
                                     pm[1][:, ls], start=True, stop=True)
                # sums replicated to partitions 0:64 / 64:128 by the ones64
                # stationary; reciprocal + multiply normalize o^T in place
                nc.tensor.matmul(ss[0:64, :], ones64[:], pm[0][:],
                                 start=True, stop=True)
                nc.tensor.matmul(ss[64:128, :], ones64[:], pm[1][:],
                                 start=True, stop=True)
                rT = mid.tile([128, T], F32, tag="rT")
                nc.vector.reciprocal_approx_fast(rT[:], ss[:])
                nc.vector.tensor_mul(oT[:, hp, g0 * 128:g0 * 128 + T], po[:], rT[:])

            # ---- E (interleaved): out projection + bias, store ----
            def emit_out(g):
                gs = slice(g * 128, (g + 1) * 128)
                ob = outp.tile([128, DIM], F32, tag="out_sb")
                for half in range(2):
                    js = slice(half * 384, (half + 1) * 384)
                    op = ps_proj.tile([128, 384], F32, tag="ps_proj")
                    for kt in range(KT):
                        nc.tensor.matmul(op[:], oT[:, kt, gs], wo_sb[:, kt, js],
                                         start=(kt == 0), stop=(kt == KT - 1))
                    nc.vector.tensor_add(ob[:, js], op[:], bias_rep[:, js])
                nc.sync.dma_start(o_d.ap()[sc, gs], ob[:])

            # Pipeline: the first two units' scores are emitted BEFORE the
            # v projection so their exp->mask chains hide under v-proj
            # matmuls; span0's groups (0..3) become out-projectable once all
            # 6 span0 pvs have run and interleave into span1's pipeline to
            # keep the PE dense through the attention phase.
            units = [(hp, span) for span in SPANS for hp in range(NHP)]
            scored = [emit_scores(*units[0]), emit_scores(*units[1])]
            emit_v_proj()
            for u in range(2, len(units)):
                emit_pv(units[u - 2][0], units[u - 2][1], scored[u - 2])
                if NHP + 2 <= u <= NHP + 5:
                    emit_out(u - NHP - 2)
                scored.append(emit_scores(*units[u]))
            emit_pv(units[-2][0], units[-2][1], scored[-2])
            emit_pv(units[-1][0], units[-1][1], scored[-1])
            for g in range(4, NG):
                emit_out(g)

    nc.compile()
    return nc


def _to_stream(x):
    """[B_LOC, 3136, d] raster -> [NTOK, d] block-major stream."""
    b, n, d = x.shape
    x = x.reshape(b, 14, 4, 14, 4, d)          # b, br, ir, bc, ic, d
    x = x.transpose(0, 1, 3, 2, 4, 5)           # b, br, bc, ir, ic, d
    return x.reshape(b * n, d)


def _from_stream(o):
    """inverse of _to_stream: [NTOK, d] -> [B_LOC, 3136, d]."""
    d = o.shape[-1]
    o = o.reshape(B_LOC, 14, 14, 4, 4, d)       # b, br, bc, ir, ic, d
    o = o.transpose(0, 1, 3, 2, 4, 5)           # b, br, ir, bc, ic, d
    return o.reshape(B_LOC, N, d)


def _make_in_maps(x, w_qkv, w_out, b_out):
    x = np.ascontiguousarray(x, dtype=np.float32)
    wq = np.ascontiguousarray(w_qkv, dtype=np.float32).astype(BFNP)
    wo = np.ascontiguousarray(w_out, dtype=np.float32).astype(BFNP)
    bo = np.ascontiguousarray(b_out, dtype=np.float32)
    in_maps = []
    for c in range(NCORES):
        xs = _to_stream(x[c * B_LOC:(c + 1) * B_LOC])      # [6272, 768]
        xT = xs.reshape(NSC, SC, DIM).transpose(0, 2, 1)   # [7, 768, 896]
        xT = np.ascontiguousarray(xT).astype(BFNP)
        in_maps.append({"x": xT, "w_qkv": wq, "w_out": wo, "b_out": bo})
    return in_maps


def kernel(x, w_qkv, w_out, b_out):
    if "nc" not in _CACHE:
        _CACHE["nc"] = _build()
    nc = _CACHE["nc"]

    in_maps = _make_in_maps(x, w_qkv, w_out, b_out)
    res = run_bass_kernel_spmd(nc, in_maps, core_ids=list(range(NCORES)))
    out = np.concatenate(
        [_from_stream(res.results[c]["o"].reshape(NTOK, DIM))
         for c in range(NCORES)], axis=0)
    return out.astype(np.float32)


# revision 19
# speedup vs baseline: 1.8315x; 1.0135x over previous
"""DiagBlockAttention Trainium2 kernel v2 (Bass/Tile, 8 NeuronCores).

Problem (hardcoded from spec nn_DiagBlockAttention):
  x[16, 3136, 768] -> qkv = x @ w_qkv -> 12 heads x 64
  block-local attention over 4x4 spatial blocks (16 tokens each),
  softmax over the 16 tokens of each block per head
  out = attn_out @ w_out + b_out

Sharding: data-parallel over batch, 2 batches per core.

v2 design (vs v1 at 990us):
- ALL matmuls bf16 (rel err ~4e-3 vs 2e-2 gate): FWL weight loads, no
  fused-f32r serial weight load, 2x DVE rates.
- x is block-permuted AND transposed to d-major ON THE HOST, so the
  stage-A PE transposes (24/chunk) vanish; x^T DMAs straight into SBUF.
- Token stream regrouped: per core 392 blocks -> 7 superchunks x 896
  tokens; each superchunk = 7 groups x 128 tokens (8 blocks). All
  attention matmuls use full 128 partitions and 128-col stationaries.
- PV matmul is swapped (stationary = v, moving = P^T) so attention
  output lands d-major; odd heads go to PSUM partitions 64:128 via the
  tile_position col-group (out.base_partition()=64). This kills the
  stage-E PE transposes too.
- Softmax sums via 1-col ones-stationary matmuls into PSUM rows 0/64;
  1/sums is partition-broadcast with a 0-stride-AP DMA, reciprocal'd
  on DVE, and multiplied into o^T d-major (normalization commutes with
  nothing else: it must happen per head before the out projection).
- Out projection consumes o^T directly; bias add doubles as the
  psum->SBUF copy.
"""
import numpy as np
import ml_dtypes
from contextlib import ExitStack

import concourse.bass as bass
import concourse.mybir as mybir
import concourse.tile as tile
from concourse import bacc
from concourse.bass_utils import run_bass_kernel_spmd

# ---- problem constants ----
B, N, DIM = 16, 3136, 768
H, DH = 12, 64
J3 = 3 * H * DH              # 2304
SCALE = DH ** -0.5           # 0.125
NCORES = 8
B_LOC = B // NCORES          # 2
NTOK = B_LOC * N             # 6272 tokens per core
NSC = 7                      # superchunks per core
SC = NTOK // NSC             # 896 tokens per superchunk
NG = SC // 128               # 7 groups of 128 tokens (8 blocks)
KT = DIM // 128              # 6 k-tiles
NHP = H // 2                 # 6 head pairs
# attention spans: groups 0..3 (512 cols) and 4..6 (384 cols)
SPANS = [(0, 4), (4, 3)]     # (first group, ngroups)
F32 = mybir.dt.float32
BF16 = mybir.dt.bfloat16
BFNP = ml_dtypes.bfloat16

_CACHE = {}


def _build():
    nc = bacc.Bacc("TRN2", target_bir_lowering=False, debug=False)

    # host-prepped inputs: x d-major bf16 per superchunk, weights bf16
    x_d = nc.dram_tensor("x", [NSC, DIM, SC], BF16, kind="ExternalInput")
    wqkv_d = nc.dram_tensor("w_qkv", [DIM, J3], BF16, kind="ExternalInput")
    wout_d = nc.dram_tensor("w_out", [DIM, DIM], BF16, kind="ExternalInput")
    bout_d = nc.dram_tensor("b_out", [DIM], F32, kind="ExternalInput")
    # output token-major (block order); host un-permutes
    o_d = nc.dram_tensor("o", [NSC, SC, DIM], F32, kind="ExternalOutput")

    with tile.TileContext(nc) as tc, ExitStack() as ctx:
        const = ctx.enter_context(tc.tile_pool(name="const", bufs=1))
        wpool = ctx.enter_context(tc.tile_pool(name="w", bufs=1))
        xin = ctx.enter_context(tc.tile_pool(name="xin", bufs=2))
        qkp_ = ctx.enter_context(tc.tile_pool(name="qkp", bufs=2))
        vap = ctx.enter_context(tc.tile_pool(name="vap", bufs=2))
        otp = ctx.enter_context(tc.tile_pool(name="otp", bufs=2))
        mid = ctx.enter_context(tc.tile_pool(name="mid", bufs=4))
        outp = ctx.enter_context(tc.tile_pool(name="outp", bufs=3))

        ps_proj = ctx.enter_context(tc.tile_pool(name="ps_proj", bufs=2, space="PSUM"))
        ps_s = ctx.enter_context(tc.tile_pool(name="ps_s", bufs=4, space="PSUM"))
        ps_pv = ctx.enter_context(tc.tile_pool(name="ps_pv", bufs=2, space="PSUM"))

        # ---- constants ----
        # 0/1 block-diag-16 mask, one [128,128] pattern repeated 4x in free
        mask = const.tile([128, 512], BF16)
        nc.gpsimd.memset(mask[:], 1.0)
        mask_v = mask[:].rearrange("p (g b i) -> p g b i", g=4, b=8)
        nc.gpsimd.affine_select(
            out=mask_v, in_=mask_v, compare_op=mybir.AluOpType.is_ge,
            fill=0.0, base=0, pattern=[[0, 4], [-16, 8], [0, 16]],
            channel_multiplier=1)
        nc.gpsimd.affine_select(
            out=mask_v, in_=mask_v, compare_op=mybir.AluOpType.is_ge,
            fill=0.0, base=15, pattern=[[0, 4], [16, 8], [0, 16]],
            channel_multiplier=-1)

        # 64 columns of ones: the sums matmul replicates the softmax
        # denominators across 64 PSUM partitions (same PE cost — the moving
        # stream is what's paid for), making the downstream reciprocal a
        # full-width DVE op with no partition broadcast needed.
        ones64 = const.tile([128, 64], BF16)
        nc.vector.memset(ones64[:], 1.0)

        # bias replicated to 128 partitions via K=1 outer-product matmul
        bias1 = const.tile([1, DIM], F32)
        nc.sync.dma_start(bias1[:], bout_d.ap().unsqueeze(0))
        ones1 = const.tile([1, 128], F32)
        nc.vector.memset(ones1[:], 1.0)
        bias_rep = const.tile([128, DIM], F32)
        for half in range(2):
            bps = ps_proj.tile([128, 384], F32, tag="ps_proj")
            nc.tensor.matmul(bps[:], ones1[:], bias1[:, half * 384:(half + 1) * 384],
                             start=True, stop=True)
            nc.vector.tensor_copy(bias_rep[:, half * 384:(half + 1) * 384], bps[:])

        # ---- weights: direct bf16 DMA, streamed in j-chunks ordered by
        # first use. The 7.2MB of weights+x is HBM-bandwidth-bound (~20us);
        # qk-proj only needs w[:, :, 0:128] to start, so chunked streaming
        # hides nearly all of it behind sc0's own matmuls.
        w_sb = wpool.tile([128, KT, J3], BF16)
        wo_sb = wpool.tile([128, KT, DIM], BF16)
        wq_src = wqkv_d.ap().rearrange("(kt p) j -> p kt j", p=128)
        wo_src = wout_d.ap().rearrange("(kt p) j -> p kt j", p=128)
        dma_engs = [nc.sync, nc.scalar, nc.gpsimd]

        def load_xT(sc):
            t = xin.tile([128, KT, SC], BF16, tag="xT")
            src = x_d.ap()[sc].rearrange("(kt p) t -> p kt t", p=128)
            nc.sync.dma_start(t[:, 0:3, :], src[:, 0:3, :])
            nc.gpsimd.dma_start(t[:, 3:6, :], src[:, 3:6, :])
            return t

        xT_next = load_xT(0)

        for i, j0 in enumerate(range(0, J3, 256)):
            dma_engs[i % 3].dma_start(
                w_sb[:, :, j0:j0 + 256], wq_src[:, :, j0:j0 + 256])
        for i, j0 in enumerate(range(0, DIM, 384)):
            dma_engs[i % 3].dma_start(
                wo_sb[:, :, j0:j0 + 384], wo_src[:, :, j0:j0 + 384])

        for sc in range(NSC):
            # ---- A: x^T (prefetched one superchunk ahead) ----
            xT = xT_next
            if sc + 1 < NSC:
                xT_next = load_xT(sc + 1)

            # ---- B: q/k projection, d-major [j, t] ----
            qk = qkp_.tile([128, H, SC], BF16, tag="qk")
            for jt in range(H):
                for half in range(2):
                    ts = slice(half * 448, (half + 1) * 448)
                    qp = ps_proj.tile([128, 448], F32, tag="ps_proj")
                    for kt in range(KT):
                        nc.tensor.matmul(
                            qp[:], w_sb[:, kt, jt * 128:(jt + 1) * 128],
                            xT[:, kt, ts],
                            start=(kt == 0), stop=(kt == KT - 1))
                    if (2 * jt + half) % 2 == 0:
                        nc.vector.tensor_copy(qk[:, jt, ts], qp[:])
                    else:
                        nc.scalar.copy(qk[:, jt, ts], qp[:])

            va = [vap.tile([128, NG, NHP, DH], BF16, tag=f"va{i}", name=f"va{i}")
                  for i in range(2)]
            oT = otp.tile([128, KT, SC], BF16, tag="oT")

            def emit_v_proj():
                # v projection, token-major, split by head parity:
                # va0[tk, g, hp, dh] = v of head 2hp; va1 = head 2hp+1
                for g in range(NG):
                    for half in range(2):
                        vp = ps_proj.tile([128, 384], F32, tag="ps_proj")
                        for kt in range(KT):
                            nc.tensor.matmul(
                                vp[:], xT[:, kt, g * 128:(g + 1) * 128],
                                w_sb[:, kt, 1536 + half * 384:1536 + (half + 1) * 384],
                                start=(kt == 0), stop=(kt == KT - 1))
                        vv = vp[:].rearrange("p (hp b d) -> p hp b d", hp=3, b=2)
                        hs = slice(3 * half, 3 * half + 3)
                        nc.vector.tensor_copy(va[0][:, g, hs, :], vv[:, :, 0, :])
                        nc.scalar.copy(va[1][:, g, hs, :], vv[:, :, 1, :])

            def emit_scores(hp, span):
                g0, ng = span
                T = ng * 128
                sp = []
                for par in range(2):
                    spt = ps_s.tile([128, T], F32, tag="ps_s", name=f"sp{par}")
                    rows = slice(64 * par, 64 * par + 64)
                    for g in range(g0, g0 + ng):
                        gs = slice(g * 128, (g + 1) * 128)
                        ls = slice((g - g0) * 128, (g - g0 + 1) * 128)
                        nc.tensor.matmul(spt[:, ls], qk[rows, 6 + hp, gs],
                                         qk[rows, hp, gs], start=True, stop=True)
                    sp.append(spt)
                pm = []
                for par in range(2):
                    pe_t = mid.tile([128, T], BF16, tag="pexp", name=f"pe{par}")
                    nc.scalar.activation(pe_t[:], sp[par][:],
                                         mybir.ActivationFunctionType.Exp,
                                         scale=SCALE)
                    pmt = mid.tile([128, T], BF16, tag="pm", name=f"pm{par}")
                    nc.vector.tensor_mul(pmt[:], pe_t[:], mask[:, 0:T])
                    pm.append(pmt)
                return pm

            def emit_pv(hp, span, pm):
                g0, ng = span
                T = ng * 128
                po = ps_pv.tile([128, T], F32, tag="ps_pv", name="po")
                ss = ps_s.tile([128, T], F32, tag="ps_s", name="ss")
                for g in range(g0, g0 + ng):
                    ls = slice((g - g0) * 128, (g - g0 + 1) * 128)
                    nc.tensor.matmul(po[0:64, ls], va[0][:, g, hp, :],
                                     pm[0][:, ls], start=True, stop=True)
                    nc.tensor.matmul(po[64:128, ls], va[1][:, g, hp, :],
                                     pm[1][:, ls], start=True, stop=True)
                # sums replicated to partitions 0:64 / 64:128 by the ones64
                # stationary; reciprocal + multiply normalize o^T in place
                nc.tensor.matmul(ss[0:64, :], ones64[:], pm[0][:],
                                 start=True, stop=True)
                nc.tensor.matmul(ss[64:128, :], ones64[:], pm[1][:],
                                 start=True, stop=True)
                rT = mid.tile([128, T], F32, tag="rT")
                nc.vector.reciprocal_approx_fast(rT[:], ss[:])
                nc.vector.tensor_mul(oT[:, hp, g0 * 128:g0 * 128 + T], po[:], rT[:])

            # ---- E (interleaved): out projection + bias, store ----
            def emit_out(g):
                gs = slice(g * 128, (g + 1) * 128)
                ob = outp.tile([128, DIM], F32, tag="out_sb")
                for half in range(2):
                    js = slice(half * 384, (half + 1) * 384)
                    op = ps_proj.tile([128, 384], F32, tag="ps_proj")
                    for kt in range(KT):
                        nc.tensor.matmul(op[:], oT[:, kt, gs], wo_sb[:, kt, js],
                                         start=(kt == 0), stop=(kt == KT - 1))
                    nc.vector.tensor_add(ob[:, js], op[:], bias_rep[:, js])
                nc.sync.dma_start(o_d.ap()[sc, gs], ob[:])

            # Pipeline: the first two units' scores are emitted BEFORE the
            # v projection so their exp->mask chains hide under v-proj
            # matmuls; span0's groups (0..3) become out-projectable once all
            # 6 span0 pvs have run and interleave into span1's pipeline to
            # keep the PE dense through the attention phase.
            units = [(hp, span) for span in SPANS for hp in range(NHP)]
            scored = [emit_scores(*units[0]), emit_scores(*units[1])]
            emit_v_proj()
            for u in range(2, len(units)):
                emit_pv(units[u - 2][0], units[u - 2][1], scored[u - 2])
                if NHP + 2 <= u <= NHP + 5:
                    emit_out(u - NHP - 2)
                scored.append(emit_scores(*units[u]))
            emit_pv(units[-2][0], units[-2][1], scored[-2])
            emit_pv(units[-1][0], units[-1][1], scored[-1])
            for g in range(4, NG):
                emit_out(g)

    nc.compile()
    return nc


def _to_stream(x):
    """[B_LOC, 3136, d] raster -> [NTOK, d] block-major stream."""
    b, n, d = x.shape
    x = x.reshape(b, 14, 4, 14, 4, d)          # b, br, ir, bc, ic, d
    x = x.transpose(0, 1, 3, 2, 4, 5)           # b, br, bc, ir, ic, d
    return x.reshape(b * n, d)


def _from_stream(o):
    """inverse of _to_stream: [NTOK, d] -> [B_LOC, 3136, d]."""
    d = o.shape[-1]
    o = o.reshape(B_LOC, 14, 14, 4, 4, d)       # b, br, bc, ir, ic, d
    o = o.transpose(0, 1, 3, 2, 4, 5)           # b, br, ir, bc, ic, d
    return o.reshape(B_LOC, N, d)


def _make_in_maps(x, w_qkv, w_out, b_out):
    x = np.ascontiguousarray(x, dtype=np.float32)
    wq = np.ascontiguousarray(w_qkv, dtype=np.float32).astype(BFNP)
    wo = np.ascontiguousarray(w_out, dtype=np.float32).astype(BFNP)
    bo = np.ascontiguousarray(b_out, dtype=np.float32)
    in_maps = []
    for c in range(NCORES):
        xs = _to_stream(x[c * B_LOC:(c + 1) * B_LOC])      # [6272, 768]
        xT = xs.reshape(NSC, SC, DIM).transpose(0, 2, 1)   # [7, 768, 896]
        xT = np.ascontiguousarray(xT).astype(BFNP)
        in_maps.append({"x": xT, "w_qkv": wq, "w_out": wo, "b_out": bo})
    return in_maps


def kernel(x, w_qkv, w_out, b_out):
    if "nc" not in _CACHE:
        _CACHE["nc"] = _build()
    nc = _CACHE["nc"]

    in_maps = _make_in_maps(x, w_qkv, w_out, b_out)
    res = run_bass_kernel_spmd(nc, in_maps, core_ids=list(range(NCORES)))
    out = np.concatenate(
        [_from_stream(res.results[c]["o"].reshape(NTOK, DIM))
         for c in range(NCORES)], axis=0)
    return out.astype(np.float32)
